# revision 38
# baseline (speedup 1.0000x reference)
"""DeepseekV3 MoE experts kernel for 8 Trainium2 NeuronCores.

Problem: every expert processes the FULL token set.
  g = x @ w_gate[e].T ; u = x @ w_up[e].T ; h = silu(g)*u
  out[e] = h @ w_down[e].T ;  concat over e -> [E*T, H]

Sharding: expert-parallel. Core c owns experts {2c, 2c+1}; hidden_states is
replicated; outputs are concatenated host-side (no on-device collectives).

Per-core compute (per expert e, with xT = x.T resident in SBUF):
  phase 1: gT[m*128:(m+1)*128, :] = wgT[:,k,mslice].T @ xT[:,k,:]  (acc over k)
  phase 2: same for uT; hT = silu(gT) * uT  (in [I, T] layout, no transposes)
  phase 3: out[mt*128.., nslice] = hT[:,k,mtslice].T @ wdT[:,k,nslice]

All matmul operands are bf16 (host-cast); PSUM accumulation is fp32; the
output is written back in bf16 and upcast to fp32 on the host (the f32
write-back measurably throttled the PE's rhs stream via HBM pressure).
Weights stream as ~0.5 MiB DMAs on the SP HWDGE ring in exact consumption
order; expert 0's gate phase runs k-outer so each weight tile is consumed
the moment it lands during DMA spin-up. The slot pools back-pressure the
stream about one expert ahead. Measured ~105us on HW (PE roofline 82us +
DMA-ramp + boot + tail).

Self-contained: shapes hardcoded; inputs are the full arrays from
setup_inputs(); returns the full [4096, 2048] fp32 output.
"""

import numpy as np
import ml_dtypes

E, T, H, I = 16, 256, 2048, 1024
N_CORES = 8
E_PER = E // N_CORES  # 2
P = 128
KO = H // P  # 16 k-chunks for phases 1/2
MO = I // P  # 8 m-chunks for phases 1/2 (= k-chunks for phase 3)
TO = T // P  # 2 m-chunks for phase 3
NS = 512  # n-slice width for phase 3
NH = H // NS  # 4

KQ = 2  # k-chunks per wg/wu eighth-tile (0.5 MiB DMAs: earlier PE start)
DQ = 2  # k-chunks per wd quarter-tile
XH = KO // 2  # k-chunks per x half-tile

_CACHE: dict = {}


def _build_program(sim_compat=False):
    # sim_compat: CoreSim lacks the Silu LUT — express silu as
    # sigmoid(g)*g with an extra DVE multiply. HW uses the fused Silu op.
    import concourse.mybir as mybir
    import concourse.tile as tile
    from concourse import bacc

    dt = mybir.dt.bfloat16
    f32 = mybir.dt.float32
    AF = mybir.ActivationFunctionType

    nc = bacc.Bacc(None, target_bir_lowering=False, debug=False)

    xT = nc.dram_tensor("xT", [P, KO, T], dt, kind="ExternalInput")[:]
    wg = nc.dram_tensor("wg", [E_PER, P, KO, I], dt, kind="ExternalInput")[:]
    wu = nc.dram_tensor("wu", [E_PER, P, KO, I], dt, kind="ExternalInput")[:]
    wd = nc.dram_tensor("wd", [E_PER, P, MO, H], dt, kind="ExternalInput")[:]
    # output in bf16: halves the output DMA bytes and the PSUM->SBUF copy
    # time; the host upcasts to fp32 (rounding adds ~1e-3 rel err, well
    # within the 2e-2 budget)
    out = nc.dram_tensor("out", [E_PER, TO, P, H], dt, kind="ExternalOutput")[:]

    with tile.TileContext(nc) as tc:
        with (
            tc.tile_pool(name="xp", bufs=9) as xp,
            tc.tile_pool(name="wp", bufs=26) as wp,
            tc.tile_pool(name="wdp", bufs=8) as wdp,
            tc.tile_pool(name="hp", bufs=2) as hp,
            tc.tile_pool(name="gp", bufs=8) as gp,
            tc.tile_pool(name="op", bufs=8) as outp,
            tc.tile_pool(name="ps", bufs=8, space="PSUM") as ps,
        ):
            # PE warm-up: matmuls on an uninitialized scratch tile with no
            # producer dependency, so they issue the moment the boot
            # barrier releases (~6.6us) and run while the first input DMAs
            # are still in flight. This flips the HAM clock gate to 8/8
            # before the first real matmul. 18 cold MMs ~= 4.4us of
            # sustained PE activity; the first real weight tiles land at
            # ~10.5us, right as the warm-up drains. (Values are garbage;
            # the PSUM tile is never read.)
            warm = xp.tile([P, T], dt, tag="warm")
            nc.vector.memset(warm[:], 0.0)
            wps = ps.tile([P, T], f32, tag="ps")
            for _ in range(18):
                nc.tensor.matmul(wps[:], warm[:, :P], warm[:], start=True, stop=True)

            xtiles = []  # (k0, kq, tile)
            wq: dict = {}  # (e, name) -> list of (k0, kq, tile)

            def issue_x(k0, kq, eng=None):
                t_ = xp.tile([P, kq, T], dt, tag="x")
                (eng or nc.sync).dma_start(t_[:], xT[:, k0 : k0 + kq, :])
                xtiles.append((k0, kq, t_))

            def xslice(k):
                for k0, kq, t_ in xtiles:
                    if k0 <= k < k0 + kq:
                        return t_[:, k - k0, :]
                raise KeyError(k)

            def issue_w(e, name, k0, kq, src, pool, eng=None):
                t_ = pool.tile([P, kq, src.shape[3]], dt, tag=pool.name)
                (eng or nc.sync).dma_start(t_[:], src[e, :, k0 : k0 + kq, :])
                wq.setdefault((e, name), []).append((k0, kq, t_))

            def wslice(e, name, k, lo, hi):
                for k0, kq, t_ in wq[(e, name)]:
                    if k0 <= k < k0 + kq:
                        return t_[:, k - k0, lo:hi]
                raise KeyError((e, name, k))

            # DMA issue order == consumption order. The leading slices are
            # extra small so the first matmul starts ~3us earlier; x is
            # woven into the first gate stream so the k-loop is never
            # input-starved. The slot pools back-pressure the stream.
            # All DMAs go through the SP HWDGE ring (nc.sync) — issuing
            # from the ACT ring measures consistently slower here. The
            # weave matches the gate k-outer consumption order: each x
            # chunk lands just before the wg chunks of the same k range.
            # All DMAs go through the SP HWDGE ring (nc.sync) in
            # consumption order — splitting the stream across the ACT
            # ring, the SWDGE (gpsimd) path, or into smaller kq=1
            # transfers all measured SLOWER; the DMA subsystem performs
            # best with one orderly FIFO of ~0.5 MiB transfers. The weave
            # matches the gate k-outer consumption order: each x chunk
            # lands just before the wg chunks of the same k range.
            issue_x(0, 2)
            issue_w(0, "wg", 0, 1, wg, wp)
            issue_w(0, "wg", 1, 1, wg, wp)
            for k0 in range(2, KO, KQ):
                issue_x(k0, KQ)
                issue_w(0, "wg", k0, KQ, wg, wp)
            for k0 in range(0, KO, KQ):
                issue_w(0, "wu", k0, KQ, wu, wp)
            for k0 in range(0, MO, DQ):
                issue_w(0, "wd", k0, DQ, wd, wdp)
            for e in range(1, E_PER):
                for k0 in range(0, KO, KQ):
                    issue_w(e, "wg", k0, KQ, wg, wp)
                for k0 in range(0, KO, KQ):
                    issue_w(e, "wu", k0, KQ, wu, wp)
                for k0 in range(0, MO, DQ):
                    issue_w(e, "wd", k0, DQ, wd, wdp)

            for e in range(E_PER):
                hT = hp.tile([P, MO, T], dt, tag="h")
                # Both experts run gate k-outer: each wg tile is consumed
                # the moment its DMA lands, so the PE never waits on a
                # straggler mid-m-loop (the whole kernel consumes in
                # stream-arrival order). Silus fire as each pg stops and
                # overlap the up phase on the ACT engine.
                pgs = [
                    ps.tile([P, T], f32, name=f"pg{e}_{m}", tag="ps")
                    for m in range(MO)
                ]
                for k in range(KO):
                    for m in range(MO):
                        nc.tensor.matmul(
                            pgs[m][:],
                            wslice(e, "wg", k, m * P, (m + 1) * P),
                            xslice(k),
                            start=(k == 0),
                            stop=(k == KO - 1),
                        )
                gss = []
                for m in range(MO):
                    gs = gp.tile([P, T], f32, name=f"gs{e}_{m}", tag="g")
                    if sim_compat:
                        nc.scalar.activation(gs[:], pgs[m][:], AF.Sigmoid)
                        nc.vector.tensor_tensor(
                            gs[:], gs[:], pgs[m][:], mybir.AluOpType.mult
                        )
                    else:
                        nc.scalar.activation(gs[:], pgs[m][:], AF.Silu)
                    gss.append(gs)
                # Up phase is k-outer for k=0..13 (again arrival-paced);
                # the last two k-steps run per-m with the DVE mult issued
                # right after each pu's stop, staggering the mult chain
                # so the down phase's h[k] reads never stall.
                pus = [
                    ps.tile([P, T], f32, name=f"pu{e}_{m}", tag="ps")
                    for m in range(MO)
                ]
                for k in range(KO - 2):
                    for m in range(MO):
                        nc.tensor.matmul(
                            pus[m][:],
                            wslice(e, "wu", k, m * P, (m + 1) * P),
                            xslice(k),
                            start=(k == 0),
                            stop=False,
                        )
                for m in range(MO):
                    for k in (KO - 2, KO - 1):
                        nc.tensor.matmul(
                            pus[m][:],
                            wslice(e, "wu", k, m * P, (m + 1) * P),
                            xslice(k),
                            start=False,
                            stop=(k == KO - 1),
                        )
                    nc.vector.tensor_tensor(
                        hT[:, m, :], gss[m][:], pus[m][:], mybir.AluOpType.mult
                    )
                for mt in range(TO):
                    for n in range(NH):
                        # The very last tile runs as two half-width
                        # accumulations so the first half's copy+DMA
                        # overlaps the second half's matmuls, shortening
                        # the end-of-kernel chain.
                        last = e == E_PER - 1 and mt == TO - 1 and n == NH - 1
                        if last:
                            for h2 in range(2):
                                lo = n * NS + h2 * (NS // 2)
                                po = ps.tile([P, NS // 2], f32, tag="ps")
                                for k in range(MO):
                                    nc.tensor.matmul(
                                        po[:],
                                        hT[:, k, mt * P : (mt + 1) * P],
                                        wslice(e, "wd", k, lo, lo + NS // 2),
                                        start=(k == 0),
                                        stop=(k == MO - 1),
                                    )
                                # final copies on DVE: don't queue behind
                                # the previous tile's ACT copy. The very
                                # last half-tile drains as two 128-col
                                # pieces so the second copy overlaps the
                                # first DMA's issue latency.
                                if h2 == 1:
                                    q = NS // 4
                                    for h4 in range(2):
                                        oq = outp.tile(
                                            [P, q], dt, name=f"oq{h4}", tag="o"
                                        )
                                        nc.vector.tensor_copy(
                                            oq[:], po[:, h4 * q : (h4 + 1) * q]
                                        )
                                        nc.sync.dma_start(
                                            out[
                                                e,
                                                mt,
                                                :,
                                                lo + h4 * q : lo + (h4 + 1) * q,
                                            ],
                                            oq[:],
                                        )
                                else:
                                    ot = outp.tile([P, NS // 2], dt, tag="o")
                                    nc.scalar.copy(ot[:], po[:])
                                    nc.sync.dma_start(
                                        out[e, mt, :, lo : lo + NS // 2], ot[:]
                                    )
                        else:
                            po = ps.tile([P, NS], f32, tag="ps")
                            for k in range(MO):
                                nc.tensor.matmul(
                                    po[:],
                                    hT[:, k, mt * P : (mt + 1) * P],
                                    wslice(e, "wd", k, n * NS, (n + 1) * NS),
                                    start=(k == 0),
                                    stop=(k == MO - 1),
                                )
                            ot = outp.tile([P, NS], dt, tag="o")
                            nc.scalar.copy(ot[:], po[:])
                            nc.sync.dma_start(
                                out[e, mt, :, n * NS : (n + 1) * NS], ot[:]
                            )

    nc.compile()
    return nc


def get_program(sim_compat=False):
    key = ("nc", sim_compat)
    if key not in _CACHE:
        _CACHE[key] = _build_program(sim_compat=sim_compat)
    return _CACHE[key]


def _prep_in_maps(hidden_states, w_gate, w_up, w_down):
    bf16 = ml_dtypes.bfloat16
    x = np.asarray(hidden_states, dtype=np.float32)
    wg = np.asarray(w_gate, dtype=np.float32)
    wu = np.asarray(w_up, dtype=np.float32)
    wd = np.asarray(w_down, dtype=np.float32)

    # xT: [H, T] -> [128, KO, T], partition p + chunk k <-> H index k*128+p
    xt = np.ascontiguousarray(
        x.T.reshape(KO, P, T).transpose(1, 0, 2).astype(bf16)
    )
    # w_gate/w_up: [E, I, H] -> per expert W.T = [H, I] -> [128, KO, I]
    wgt = np.ascontiguousarray(
        wg.transpose(0, 2, 1).reshape(E, KO, P, I).transpose(0, 2, 1, 3).astype(bf16)
    )
    wut = np.ascontiguousarray(
        wu.transpose(0, 2, 1).reshape(E, KO, P, I).transpose(0, 2, 1, 3).astype(bf16)
    )
    # w_down: [E, H, I] -> per expert W.T = [I, H] -> [128, MO, H]
    wdt = np.ascontiguousarray(
        wd.transpose(0, 2, 1).reshape(E, MO, P, H).transpose(0, 2, 1, 3).astype(bf16)
    )

    in_maps = []
    for c in range(N_CORES):
        sl = slice(c * E_PER, (c + 1) * E_PER)
        in_maps.append(
            {
                "xT": xt,
                "wg": np.ascontiguousarray(wgt[sl]),
                "wu": np.ascontiguousarray(wut[sl]),
                "wd": np.ascontiguousarray(wdt[sl]),
            }
        )
    return in_maps


def kernel(hidden_states, w_gate, w_up, w_down, _trace=False, _trace_kwargs=None):
    from concourse.bass_utils import run_bass_kernel_spmd

    nc = get_program()
    in_maps = _prep_in_maps(hidden_states, w_gate, w_up, w_down)
    kwargs = {}
    if _trace:
        kwargs = dict(trace=True, **(_trace_kwargs or {}))
    res = run_bass_kernel_spmd(nc, in_maps, core_ids=list(range(N_CORES)), **kwargs)
    out = np.concatenate(
        [
            res.results[c]["out"].reshape(E_PER * T, H).astype(np.float32)
            for c in range(N_CORES)
        ],
        axis=0,
    )
    if _trace:
        _CACHE["last_results"] = res
    return out

